# revision 34
# baseline (speedup 1.0000x reference)
"""Llama attention layer (B=2, S=2048, D=2048, H=16, HD=128, RoPE, causal)
on 8 Trainium2 NeuronCores.

Sharding: core c -> (batch b = c//4, head group g = c%4 of 4 heads).
Each core computes q/k/v projections for its 512 columns of wq/wk/wv,
RoPE, causal attention for its 4 heads, and the out-projection against
its 512 rows of wo (a partial sum over head groups). The host sums the
4 partials per batch and stacks the 2 batches.

All device matmuls run in bf16 with fp32 PSUM accumulation. Softmax is
computed without max-subtraction (scores here are bounded ~|9|).

Perf structure (~353us naive -> ~315us; PE matmul-column floor ~273us
plus ~11us fixed NEFF preamble/teardown):
- xT is packed m-major on the host AND in SBUF so every DMA line is
  contiguous; K/Q-projection matmuls read it through a strided
  [128,4,128] rhs AP.
- Input DMAs split across both hardware DGE queues, k-sliced so the
  first V matmuls start during the stream; the leading slices are
  halved because the queues ramp from ~50GB/s (and 1KB-line DMAs are
  ~5x slower than 4KB-line ones, so wv moves in 4-slice groups).
- Pools are few (tags inside shared pools): every pool open/close is
  an all-engine barrier round, and the closes stack in the epilogue.
- V runs k-outer over waves of 4 PSUM banks; warmup matmuls on zeros
  pin the PE p-state while the first DMAs land.
- RoPE drains each projection psum via THREE ACT copies (straight +
  two swapped halves); the multiplies then run as 2x-rate bf16 SBUF
  ops on DVE, halving DVE cost vs reading fp32 psum at 1x.
- qt=0 attention is sliced into steps embedded in the Q-projection
  loop (attn@V one step behind scores+exp), so its ACT latency hides
  under projection matmuls; the last Q head runs as two 256-col
  halves so only half a rope drain is left at the phase boundary.
- Softmax denominator: DVE accumulates exp-chunk pairs into a running
  total (tail adds narrowed to causally-live columns); ONE ones-matmul
  per (qt,h) does the partition reduction, deferred into the next
  attend (pending_fin) so the in-order PE never waits on the DVE
  chain.
- Phase-2/3: attends run a depth-4 software pipeline (chunk m's
  attn@V emitted after chunk m+4's scores) interleaved with the
  previous q-tile's out-projection rows; attn@V is one matmul per
  chunk (stop only on the last, skip_group_check).
- Output rows stage in [128,2048] tiles -> ONE DMA per row block
  (descriptor gen is ~0.6us each); casts alternate DVE/ACT; the final
  emits alternate onto the idle scores banks and the last row block
  DMAs per-slice so the tail transfer starts ASAP.
- Output partials are written bf16 (host accumulates in fp32).
"""

import os
import sys

import numpy as np
import ml_dtypes

if "/opt/trn_rl_repo" not in sys.path:
    sys.path.insert(0, "/opt/trn_rl_repo")

import concourse.bass as bass  # noqa: E402
import concourse.mybir as mybir  # noqa: E402
import concourse.bacc as bacc  # noqa: E402
import concourse.tile as tile  # noqa: E402

BF16 = ml_dtypes.bfloat16

B, S, D, H = 2, 2048, 2048, 16
HD = D // H            # 128, head dim
G = 4                  # head groups (cores per batch)
NH = H // G            # 4 heads per core
DG = NH * HD           # 512, per-core head width
P = 128
KO = D // P            # 16 k-subtiles over D
NKT = S // P           # 16 key chunks of 128
NQT = S // 512         # 4 q tiles of 512
QT = 512
ROPE_THETA = 10000.0
SCALE = 1.0 / float(np.sqrt(HD))

N_CORES = 8

_BUILT = None  # (nc,) cache


def build_module():
    fp32 = mybir.dt.float32
    bf16 = mybir.dt.bfloat16

    nc = bacc.Bacc("TRN2", target_bir_lowering=False, debug=False,
                   num_devices=N_CORES, num_swdge_queues=2)

    xT = nc.dram_tensor("xT", [P, NKT, KO, P], bf16, kind="ExternalInput")
    wq = nc.dram_tensor("wq", [P, KO, DG], bf16, kind="ExternalInput")
    wk = nc.dram_tensor("wk", [P, KO, DG], bf16, kind="ExternalInput")
    wv = nc.dram_tensor("wv", [P, KO, DG], bf16, kind="ExternalInput")
    wo = nc.dram_tensor("wo", [P, NH, D], bf16, kind="ExternalInput")
    cosT = nc.dram_tensor("cosT", [P, S], bf16, kind="ExternalInput")
    sinT = nc.dram_tensor("sinT", [P, S], bf16, kind="ExternalInput")
    maskT = nc.dram_tensor("maskT", [P, P], bf16, kind="ExternalInput")
    out = nc.dram_tensor("out", [P, NKT, D], bf16, kind="ExternalOutput")

    Exp = mybir.ActivationFunctionType.Exp

    with tile.TileContext(nc) as tc:
        # Pools are deliberately few: every tile_pool open/close costs an
        # all-engine barrier round (~0.5us each, and the closes stack up in
        # the program epilogue), so logical groups live as TAGS inside a
        # shared pool instead of separate pools.
        with tc.tile_pool(name="big", bufs=1) as big:
            ones = big.tile([P, P], bf16)
            nc.vector.memset(ones, 1.0)
            warm = big.tile([1, 1], fp32)

            qT_sb = big.tile([P, NH, S], bf16)   # per head: [HD, S]
            kT_sb = big.tile([P, NH, S], bf16)
            v_sb = big.tile([P, NKT, DG], bf16)  # [key%128, keychunk, dg]
            wo_sb = big.tile([P, NH, D], bf16)
            mask_sb = big.tile([P, P], bf16)
            ao0_sb = big.tile([P, NH, QT], bf16)  # qt=0 attention out

            # --- deferred-finalize attention --------------------------
            # Each attend's denominator is built entirely on DVE (pair adds
            # + a running total); the partition reduction is ONE ones-matmul
            # per (qt, h), deferred until after more independent PE work has
            # been queued so the in-order PE never waits on the DVE chain.
            pending_fin = []  # [(total, ps_o, dst)] awaiting normalize

            def flush_fin(pool_sum, pool_ep, ep_bufs):
                while pending_fin:
                    total, ps_o_p, dst_p = pending_fin.pop()
                    ps_sum = pool_sum.tile([P, QT], fp32, tag="ps_sum",
                                           bufs=1)
                    nc.tensor.matmul(ps_sum, ones, total,
                                     start=True, stop=True)
                    rec = pool_ep.tile([P, QT], fp32, tag="rec",
                                       bufs=ep_bufs)
                    nc.vector.reciprocal_approx_fast(rec, ps_sum)
                    nc.vector.tensor_mul(dst_p, ps_o_p, rec)

            def attend_scores(qt, h, m, st, pools):
                """Scores matmul + exp for key-chunk m of (qt, h)."""
                (pool_s, s_bufs, pool_o, o_bufs, pool_ax, ax_bufs,
                 pool_axp, axp_bufs, pool_tot, tot_bufs) = pools
                o = m - qt * 4
                colo = max(0, o) * P
                ps_s = pool_s.tile([P, QT], fp32, tag="ps_s", bufs=s_bufs)
                nc.tensor.matmul(ps_s[:, colo:],
                                 kT_sb[:, h, m * P:(m + 1) * P],
                                 qT_sb[:, h,
                                       qt * QT + colo:(qt + 1) * QT],
                                 start=True, stop=True)
                ax = pool_ax.tile([P, QT], bf16, tag="ax", bufs=ax_bufs)
                if colo:
                    # zero the causally-dead prefix so the denominator adds
                    # see zeros there
                    nc.gpsimd.memset(ax[:, 0:colo], 0.0)
                nc.scalar.activation(ax[:, colo:], ps_s[:, colo:],
                                     Exp, scale=SCALE)
                st[("ax", m)] = ax

            def attend_av(qt, h, m, st, pools):
                """Mask + attn@V + denominator adds for key-chunk m."""
                (pool_s, s_bufs, pool_o, o_bufs, pool_ax, ax_bufs,
                 pool_axp, axp_bufs, pool_tot, tot_bufs) = pools
                if m == 0:
                    st["ps_o"] = pool_o.tile([P, QT], fp32, tag="ps_o",
                                             bufs=o_bufs, name="ps_o")
                ps_o = st["ps_o"]
                ax = st.pop(("ax", m))
                o = m - qt * 4
                colo = max(0, o) * P
                if o >= 0:
                    # triangular mask on the 128 cols straddling the diagonal
                    nc.vector.tensor_mul(ax[:, colo:colo + P],
                                         ax[:, colo:colo + P], mask_sb)
                # attn @ V, narrowed to the causally-live columns. Only
                # the attend's LAST chunk carries stop=True; the per-region
                # accumulation groups stay open (skip_group_check) -- the
                # hardware doesn't use stop, and the normalize read is
                # dependency-ordered after every AV matmul anyway. This
                # avoids splitting each diagonal chunk into two matmuls
                # (每 matmul costs ~50ns of LDWEIGHTS/dispatch).
                vsl = v_sb[:, m, h * HD:(h + 1) * HD]
                n_kt = 4 * (qt + 1)
                nc.tensor.matmul(ps_o[:, colo:], vsl, ax[:, colo:],
                                 start=(m == 0), stop=(m == n_kt - 1),
                                 skip_group_check=True)
                if m % 2 == 0:
                    st["ax_prev"] = ax
                else:
                    pair = pool_axp.tile([P, QT], bf16, tag="axp",
                                         bufs=axp_bufs)
                    # cp: both chunks of the pair are zero left of the
                    # earlier chunk's causal offset, so the running-total
                    # add can skip that prefix (the pair add itself must
                    # stay full-width only when the pair feeds the m==3
                    # fresh-total add)
                    cp = max(0, m - 1 - qt * 4) * P
                    if m == 1:
                        nc.vector.tensor_add(pair, st["ax_prev"], ax)
                        st["pair_first"] = pair
                    elif m == 3:
                        nc.vector.tensor_add(pair, st["ax_prev"], ax)
                        tot = pool_tot.tile([P, QT], bf16, tag="tot",
                                            bufs=tot_bufs, name="tot")
                        nc.vector.tensor_add(tot, st["pair_first"], pair)
                        st["total"] = tot
                    else:
                        nc.vector.tensor_add(pair[:, cp:],
                                             st["ax_prev"][:, cp:],
                                             ax[:, cp:])
                        nc.vector.tensor_add(st["total"][:, cp:],
                                             st["total"][:, cp:],
                                             pair[:, cp:])

            def attend_chunk(qt, h, m, st, pools):
                """One key-chunk of causal attention for (qt, h)."""
                attend_scores(qt, h, m, st, pools)
                attend_av(qt, h, m, st, pools)

            # ---------------- phase 1: projections + RoPE ----------------
            with tc.tile_pool(name="w_pool", bufs=1) as w_pool, \
                 tc.tile_pool(name="ps1", bufs=1, space="PSUM") as ps1:
                # DMA order matters: wv first (V-loop gate), then xT in
                # m-major column blocks (the dram layout is packed so block m
                # is contiguous) so V m-group m only waits for its own block,
                # then the K/Q-phase tensors, then phase-2/3 tensors.
                wv_sb = w_pool.tile([P, KO, DG], bf16)
                # m-major like the DRAM packing: per-block DMA is contiguous
                # (4KB/partition). A k-major SBUF layout would make the block
                # DMA scatter 256B lines, which runs ~7x slower.
                xT_sb = w_pool.tile([P, NKT, KO, P], bf16)
                wk_sb = w_pool.tile([P, KO, DG], bf16)
                cos_sb = w_pool.tile([P, S], bf16)
                sin_sb = w_pool.tile([P, S], bf16)
                wq_sb = w_pool.tile([P, KO, DG], bf16)
                # Descriptor generation costs ~0.6us of sequencer time per
                # dma_start, so split the input stream over BOTH hardware DGE
                # queues. wv is the V-phase gate, so it is split ACROSS both
                # queues to land as fast as possible: singles k0-3 lead the
                # SP queue (which starts pumping ~1.8us before the Activation
                # queue), groups k4-15 lead the Activation queue. xT blocks
                # follow on SP; the K/Q/phase-2 tensors follow on Activation.
                # The DMA stream ramps slowly (~75GB/s for the first
                # ~5us), so the FIRST slice on each queue is kept small (wv
                # k0 single / k1-3 triple) to unblock the first V matmuls;
                # later wv slices ride as 4-slice groups. xT blocks are
                # split across BOTH queues (q4 alone measured ~160GB/s while
                # q10 sat idle after the weights), interleaved so each V
                # wave's blocks land just ahead of its matmuls.
                # the queue ramps from ~50GB/s, so the leading slices
                # are halved: smaller first transfers unblock the first V
                # matmuls sooner
                nc.sync.dma_start(wv_sb[:, 0, 0:256], wv.ap()[:, 0, 0:256])
                nc.sync.dma_start(wv_sb[:, 0, 256:], wv.ap()[:, 0, 256:])
                for m in (0, 1, 2, 3):
                    nc.sync.dma_start(xT_sb[:, m, 0:8], xT.ap()[:, m, 0:8])
                    nc.sync.dma_start(xT_sb[:, m, 8:], xT.ap()[:, m, 8:])
                for m in (6, 7, 10, 11, 14, 15):
                    nc.sync.dma_start(xT_sb[:, m], xT.ap()[:, m])
                nc.scalar.dma_start(wv_sb[:, 1:4, :], wv.ap()[:, 1:4, :])
                for ks_ in range(4, KO, 4):
                    nc.scalar.dma_start(wv_sb[:, ks_:ks_ + 4, :],
                                        wv.ap()[:, ks_:ks_ + 4, :])
                nc.scalar.dma_start(xT_sb[:, 4], xT.ap()[:, 4])
                nc.scalar.dma_start(xT_sb[:, 5], xT.ap()[:, 5])
                nc.scalar.dma_start(wk_sb, wk.ap())
                nc.scalar.dma_start(xT_sb[:, 8], xT.ap()[:, 8])
                nc.scalar.dma_start(xT_sb[:, 9], xT.ap()[:, 9])
                nc.scalar.dma_start(cos_sb, cosT.ap())
                nc.scalar.dma_start(sin_sb, sinT.ap())
                nc.scalar.dma_start(xT_sb[:, 12], xT.ap()[:, 12])
                nc.scalar.dma_start(xT_sb[:, 13], xT.ap()[:, 13])
                nc.scalar.dma_start(wq_sb, wq.ap())
                nc.scalar.dma_start(mask_sb, maskT.ap())
                nc.scalar.dma_start(wo_sb, wo.ap())
                # dummy exp so the ACT Exp table load (~1.3us on the ACT
                # sequencer) happens AFTER the Activation-queue descriptors
                # are generated -- issuing it earlier delays the weight
                # stream's first transfer
                nc.scalar.activation(warm, ones[0:1, 0:1],
                                     mybir.ActivationFunctionType.Exp)

                # attends for q-tile 0 run EMBEDDED in the Q-projection loop
                # below (see a0_steps); their tiles live in phase-1 pools
                a0_pools = (ps1, 2, ps1, 1, w_pool, 3, w_pool, 2, w_pool, 2)

                # warmup rhs: borrow an "ax" ring slot (zeroed; the ring
                # recycles it for attention later)
                warm_mm = w_pool.tile([P, QT], bf16, tag="ax", bufs=3,
                                      name="warm_mm")
                nc.gpsimd.memset(warm_mm, 0.0)

                # PE p-state warmup: the tensor engine clocks up only after
                # ~3us of continuous execution, so chew on zeros while the
                # first wv/xT DMAs land -- the first real matmuls then run at
                # full clock instead of half.
                ps_w = ps1.tile([P, QT], fp32, tag="psv", bufs=4)
                for r in range(14):
                    nc.tensor.matmul(ps_w, ones, warm_mm,
                                     start=(r == 0), stop=(r == 13))

                # V: [keys, dg] natural layout, keychunk tiles of 128.
                # k-OUTER waves of 4 m-groups (4 psum banks): each wv k-slice
                # unlocks 4 matmuls, so the PE ramps as the k-sliced wv DMA
                # trickles in instead of waiting for all of wv.
                for mw, nwv in ((0, 4), (4, 4), (8, 4), (12, 4)):
                    pss = [ps1.tile([P, DG], fp32, tag="psv",
                                    name=f"psv{i}", bufs=4)
                           for i in range(nwv)]
                    for k in range(KO):
                        for i in range(nwv):
                            if mw == 0 and k == 0:
                                # wave 0's k=0 split by dg-half to match the
                                # halved wv k0 DMA (start=True on the first
                                # half clears the bank; the second half
                                # lands on unwritten psum, so overwrite)
                                nc.tensor.matmul(
                                    pss[i][:, 0:256],
                                    xT_sb[:, mw + i, k, :],
                                    wv_sb[:, k, 0:256],
                                    start=True, stop=False)
                                nc.tensor.matmul(
                                    pss[i][:, 256:],
                                    xT_sb[:, mw + i, k, :],
                                    wv_sb[:, k, 256:],
                                    start=False, stop=False)
                            else:
                                nc.tensor.matmul(
                                    pss[i], xT_sb[:, mw + i, k, :],
                                    wv_sb[:, k, :],
                                    start=(k == 0 and mw != 0),
                                    stop=(k == KO - 1))
                    for i in range(nwv):
                        nc.vector.tensor_copy(v_sb[:, mw + i, :], pss[i])

                # qt=0 attention, sliced into steps interleaved with the
                # Q-projection matmuls: each chunk's attn@V runs one step
                # AFTER its scores+exp, so the exp latency hides under the
                # projection matmuls between steps and the in-order PE never
                # stalls. Head 3 is held back (gate 99) to run AFTER the Q
                # loop, covering the last head-pair's serialized RoPE drain
                # (~4us of DVE) that otherwise gates the phase transition.
                a0_steps = []  # [(min_nt2, closure)]
                for h0 in range(NH):
                    st0 = {}
                    gate = h0 // 2 + 1

                    def mk_sc(h0, m, st0):
                        return lambda: attend_scores(0, h0, m, st0, a0_pools)

                    def mk_av_sc(h0, m, st0):
                        def f():
                            attend_av(0, h0, m - 1, st0, a0_pools)
                            attend_scores(0, h0, m, st0, a0_pools)
                        return f

                    def mk_av(h0, m, st0):
                        return lambda: attend_av(0, h0, m, st0, a0_pools)

                    a0_steps.append((gate, mk_sc(h0, 0, st0)))
                    for m in range(1, 4):
                        a0_steps.append((gate, mk_av_sc(h0, m, st0)))
                    a0_steps.append((gate, mk_av(h0, 3, st0)))

                    def mk_fin(h0=h0, st0=st0):
                        def fin():
                            pending_fin.append(
                                (st0["total"], st0["ps_o"],
                                 ao0_sb[:, h0, :]))
                            flush_fin(ps1, w_pool, 1)
                        return fin
                    a0_steps.append((gate, mk_fin(h0, st0)))
                a0_next = 0

                # K then Q: [HD, S] transposed layout + RoPE.
                # Heads processed in pairs so the two psum tags can be
                # double-buffered (2 tags x 2 bufs) -- RoPE of one pair
                # overlaps the matmuls of the next.
                def rope(ps, dst, sl, w=QT):
                    # rope: dst = ps * cos + swap(ps) * sin_signed.
                    # The psum drains via THREE ACT copies (straight qb +
                    # two swapped halves into qsw): fp32-psum reads on DVE
                    # run at 1x (~660ns) while the bf16 SBUF muls below run
                    # at 2x (~330ns), so pushing all psum reads to the idle
                    # ACT engine halves the DVE rope cost and drains the
                    # bank sooner.
                    qb = w_pool.tile([P, QT], bf16, tag="qb",
                                     bufs=4, name="qb")[:, 0:w]
                    nc.scalar.copy(qb, ps)
                    qsw = w_pool.tile([P, QT], bf16, tag="qsw",
                                      bufs=2, name="qsw")[:, 0:w]
                    nc.scalar.copy(qsw[0:64], ps[64:128])
                    nc.scalar.copy(qsw[64:128], ps[0:64])
                    tmp = w_pool.tile([P, QT], bf16, tag="tmp",
                                      bufs=4, name="tmp")[:, 0:w]
                    nc.vector.tensor_mul(tmp, qsw, sin_sb[:, sl])
                    nc.vector.tensor_mul(dst, qb, cos_sb[:, sl])
                    nc.vector.tensor_add(dst, dst, tmp)

                for which, w_sb, dstT in (("k", wk_sb, kT_sb),
                                          ("q", wq_sb, qT_sb)):
                    for nt2 in range(2 * NQT):
                        nt, hp = divmod(nt2, 2)
                        sl = slice(nt * QT, (nt + 1) * QT)
                        heads = (2 * hp, 2 * hp + 1)
                        if which == "q" and nt2 == 2 * NQT - 1:
                            # LAST iteration: heads run sequentially so head
                            # A's rope drain (ACT+DVE, ~2.7us) overlaps head
                            # B's matmuls; only head B's rope is left naked
                            # at the phase boundary (the all-engine pool
                            # barrier waits for it).
                            h = heads[0]
                            psl = ps1.tile([P, QT], fp32, tag="psv",
                                           name=f"psl{h}", bufs=4)
                            for k in range(KO):
                                nc.tensor.matmul(
                                    psl,
                                    w_sb[:, k, h * HD:(h + 1) * HD],
                                    xT_sb[:, nt * 4:(nt + 1) * 4, k, :],
                                    start=(k == 0), stop=(k == KO - 1))
                                if k % 4 == 3 and a0_next < len(a0_steps):
                                    a0_steps[a0_next][1]()
                                    a0_next += 1
                            rope(psl, dstT[:, h, sl], sl)
                            # very last head: two 256-col halves, each roped
                            # as soon as its half-psum closes -- the first
                            # half's rope drains under the second half's
                            # matmuls, halving the naked tail drain
                            h = heads[1]
                            for half in range(2):
                                mws = nt * 4 + 2 * half
                                hsl = slice(nt * QT + 256 * half,
                                            nt * QT + 256 * (half + 1))
                                psh = ps1.tile([P, 256], fp32, tag="psv",
                                               name="psh", bufs=4)
                                for k in range(KO):
                                    nc.tensor.matmul(
                                        psh,
                                        w_sb[:, k, h * HD:(h + 1) * HD],
                                        xT_sb[:, mws:mws + 2, k, :],
                                        start=(k == 0), stop=(k == KO - 1))
                                rope(psh, dstT[:, h, hsl], hsl, w=256)
                            continue
                        # share the "psv" tag (banks 0-3) so the projections
                        # stay within 4 psum banks, leaving 4-7 free for the
                        # embedded attends
                        pss = {}
                        for h in heads:
                            pss[h] = ps1.tile([P, QT], fp32, tag="psv",
                                              name=f"psp{h}", bufs=4)
                        # rhs: the nt-th 512 queries = xT m-blocks 4nt..4nt+3
                        # at fixed k -- a strided [128, 4, 128] AP
                        for k in range(KO):
                            for h in heads:
                                nc.tensor.matmul(
                                    pss[h], w_sb[:, k, h * HD:(h + 1) * HD],
                                    xT_sb[:, nt * 4:(nt + 1) * 4, k, :],
                                    start=(k == 0), stop=(k == KO - 1))
                            if which == "q" and k % 4 == 3:
                                # attend step slot (only once the needed qT
                                # head has been roped: min_nt2 gate)
                                if (a0_next < len(a0_steps)
                                        and a0_steps[a0_next][0] <= nt2):
                                    a0_steps[a0_next][1]()
                                    a0_next += 1
                        for h in heads:
                            rope(pss[h], dstT[:, h, sl], sl)

                # any attend steps not yet drained (shouldn't happen: 20
                # steps vs 28 slots)
                while a0_next < len(a0_steps):
                    a0_steps[a0_next][1]()
                    a0_next += 1

            # ---------------- phases 2+3 ----------------
            with tc.tile_pool(name="big2", bufs=1) as big2, \
                 tc.tile_pool(name="ps2", bufs=1, space="PSUM") as ps2:
                aoT_sb = big2.tile([P, NH, S], bf16)  # attention out^T
                # pre-create the psum tags whose first use comes late, so
                # the scores tag (first phase-2 PE writes) lands on the
                # banks the embedded attends freed earliest
                _d0 = ps2.tile([P, QT], fp32, tag="ps_out", bufs=2,
                               name="d0")
                _d1 = ps2.tile([P, QT], fp32, tag="ps_sum", bufs=1,
                               name="d1")
                a2_pools = (ps2, 3, ps2, 2, big2, 20, big2, 6, big2, 3)

                def emit_outproj(qo, split_dma=False, flush_mid=False,
                                 deep_ps=False):
                    # one [128, 2048] staging tile per query-row block: a
                    # single contiguous output DMA instead of four
                    # (descriptor generation is ~0.6us of sequencer time
                    # each). Casts alternate DVE/ACT except on emits 11-12,
                    # where ACT is still busy with the last attend's exps --
                    # those go DVE-only.
                    ob = big2.tile([P, D], bf16, tag="ob", bufs=3)
                    for n in range(D // QT):
                        nsl = slice(n * QT, (n + 1) * QT)
                        if deep_ps and n % 2 == 1:
                            # after the last attend the scores banks sit
                            # idle; alternating onto them doubles the
                            # outproj bank-recycle distance so the matmuls
                            # stop waiting on the casts
                            ps = ps2.tile([P, QT], fp32, tag="ps_s", bufs=3)
                        else:
                            ps = ps2.tile([P, QT], fp32, tag="ps_out",
                                          bufs=2)
                        for h in range(NH):
                            if qo < 4:
                                lhs = ao0_sb[:, h, qo * P:(qo + 1) * P]
                            else:
                                lhs = aoT_sb[:, h, qo * P:(qo + 1) * P]
                            nc.tensor.matmul(
                                ps, lhs, wo_sb[:, h, nsl],
                                start=(h == 0), stop=(h == NH - 1))
                        if n == 3 and flush_mid:
                            # the last attend's finalize, emitted here so
                            # its DVE normalize queues ahead of this emit's
                            # remaining casts (the following emits read the
                            # normalized aoT)
                            flush_fin(ps2, big2, 2)
                        if n % 2 == 1 or (flush_mid and n >= 1):
                            nc.scalar.copy(ob[:, nsl], ps)
                        else:
                            nc.vector.tensor_copy(ob[:, nsl], ps)
                        if split_dma:
                            # last row block: per-slice DMAs (alternating
                            # queues for parallel descriptor generation) so
                            # the final transfer starts right after its own
                            # cast
                            q = nc.sync if n % 2 == 0 else nc.scalar
                            q.dma_start(out.ap()[:, qo, nsl], ob[:, nsl])
                    if not split_dma:
                        nc.sync.dma_start(out.ap()[:, qo, :], ob)

                # attention for q-tiles 1-3, interleaved with the
                # out-projection rows the previous q-tile unblocked (those
                # matmuls have no ACT dependency and fill the exp-latency
                # bubbles). Each attend's finalize is flushed inside the
                # NEXT attend's chunk loop (pending_fin).
                for qt in range(1, NQT):
                    qsl = slice(qt * QT, (qt + 1) * QT)
                    for h in range(NH):
                        st = {}
                        n_kt = 4 * (qt + 1)
                        # depth-4 software pipeline: chunk m's attn@V is
                        # emitted after chunk m+4's scores, giving the exp
                        # ~1.1us of PE slack so the in-order PE never waits
                        # on the ACT chain even when casts queue ahead of
                        # the exps on ACT
                        for m in range(n_kt + 4):
                            if m == 5:
                                # finalize the PREVIOUS attend here: by
                                # chunk 5 its DVE pair/total chain (which
                                # competes with the interleaved emit's
                                # casts) is reliably drained, so the
                                # ones-matmul no longer stalls the PE
                                flush_fin(ps2, big2, 2)
                            if m < n_kt:
                                attend_scores(qt, h, m, st, a2_pools)
                            if m >= 4:
                                attend_av(qt, h, m - 4, st, a2_pools)
                        pending_fin.append(
                            (st["total"], st["ps_o"], aoT_sb[:, h, qsl]))
                        emit_outproj(4 * (qt - 1) + h,
                                     flush_mid=(qt == NQT - 1
                                                and h == NH - 1))

                # the last q-tile's rows have nothing to hide behind
                for qo in range(4 * (NQT - 1), 4 * NQT):
                    emit_outproj(qo, split_dma=(qo == 4 * NQT - 1),
                                 deep_ps=True)

    nc.compile()
    return nc


def _rope_tables():
    inv_freq = 1.0 / (ROPE_THETA ** (np.arange(0, HD, 2, dtype=np.float64) / HD))
    pos = np.arange(S, dtype=np.float64)
    freqs = np.outer(pos, inv_freq)                    # [S, HD/2]
    emb = np.concatenate([freqs, freqs], axis=-1)      # [S, HD]
    cos = np.cos(emb).T.astype(BF16)                   # [HD, S]
    sin = np.sin(emb).T.astype(np.float32)
    sin[: HD // 2] *= -1.0                             # fold rotate_half sign
    return cos, sin.astype(BF16)


def _pack_kd(a):
    """[D, N] -> [P, D//P, N] with d = ko*P + p."""
    d, n = a.shape
    return np.ascontiguousarray(
        a.reshape(d // P, P, n).transpose(1, 0, 2)).astype(BF16)


def _pack_xT(xb):
    """x[b] [S, D] -> [P, NKT, KO, P] m-major so each 128-col block of x^T
    is one contiguous DMA."""
    t = _pack_kd(np.ascontiguousarray(xb.T))           # [P, KO, S]
    return np.ascontiguousarray(
        t.reshape(P, KO, NKT, P).transpose(0, 2, 1, 3))


def make_in_maps(x, wq, wk, wv, wo):
    cosT, sinT = _rope_tables()
    i = np.arange(P)[:, None]
    j = np.arange(P)[None, :]
    mask = (i <= j).astype(BF16)

    xT_packed = [_pack_xT(x[b]) for b in range(B)]
    in_maps = []
    for c in range(N_CORES):
        b, g = divmod(c, G)
        gsl = slice(g * DG, (g + 1) * DG)
        in_maps.append({
            "xT": xT_packed[b],
            "wq": _pack_kd(wq[:, gsl]),
            "wk": _pack_kd(wk[:, gsl]),
            "wv": _pack_kd(wv[:, gsl]),
            "wo": _pack_kd(np.ascontiguousarray(wo[gsl, :])),
            "cosT": cosT,
            "sinT": sinT,
            "maskT": mask,
        })
    return in_maps


def assemble_output(results):
    """results: list of 8 dicts with 'out' [P, NKT, D] bf16 partials."""
    full = np.empty((B, S, D), dtype=np.float32)
    for b in range(B):
        acc = None
        for g in range(G):
            r = results[b * G + g]["out"].astype(np.float32)
            part = r.transpose(1, 0, 2).reshape(S, D)
            acc = part if acc is None else acc + part
        full[b] = acc
    return full


def _get_module():
    global _BUILT
    if _BUILT is None:
        _BUILT = build_module()
    return _BUILT


def _install_trace_shim():
    """This image's antenv lacks axon_hooks; provide the NTFF profile hook
    via ctypes so trace=True (or BASS_TRACE=1) works instead of crashing,
    and skip the artifact bucket upload."""
    try:
        import antenv.axon_hooks  # noqa: F401
        return
    except ImportError:
        pass
    import types
    import ctypes
    import contextlib

    so_path = "/opt/axon/libaxon_pjrt.so"
    mod = types.ModuleType("antenv.axon_hooks")
    try:
        lib = ctypes.CDLL(so_path)
        lib.axon_start_nrt_profile.argtypes = [
            ctypes.POINTER(ctypes.c_int64), ctypes.c_size_t]
        lib.axon_start_nrt_profile.restype = ctypes.c_int64
        lib.axon_stop_nrt_profile.argtypes = [ctypes.c_char_p]
        lib.axon_stop_nrt_profile.restype = ctypes.c_int64

        @contextlib.contextmanager
        def _hook(output_dir, device_ids):
            import jax
            jax.devices()
            if device_ids:
                ids = (ctypes.c_int64 * len(device_ids))(*device_ids)
                rc = lib.axon_start_nrt_profile(ids, len(device_ids))
            else:
                rc = lib.axon_start_nrt_profile(None, 0)
            if rc != 0:
                raise RuntimeError(f"axon_start_nrt_profile rc={rc}")
            try:
                yield
            finally:
                lib.axon_stop_nrt_profile(str(output_dir).encode())

        mod.get_axon_ntff_profile_hook = lambda: _hook
    except OSError:
        mod.get_axon_ntff_profile_hook = lambda: None
    mod.set_axon_ntff_profile_hook = lambda h: None
    sys.modules["antenv.axon_hooks"] = mod

    from concourse import bass_utils
    bass_utils.upload_artifacts = lambda tmpdir: tmpdir


def run_on_hw(in_maps, trace=False, trace_cores=None):
    _install_trace_shim()
    from concourse import bass_utils
    nc = _get_module()
    return bass_utils.run_bass_kernel_spmd(
        nc, in_maps, core_ids=list(range(N_CORES)),
        trace=trace, trace_cores=trace_cores)


def kernel(x, wq, wk, wv, wo):
    x = np.asarray(x, dtype=np.float32)
    wq = np.asarray(wq, dtype=np.float32)
    wk = np.asarray(wk, dtype=np.float32)
    wv = np.asarray(wv, dtype=np.float32)
    wo = np.asarray(wo, dtype=np.float32)
    in_maps = make_in_maps(x, wq, wk, wv, wo)
    res = run_on_hw(in_maps, trace=False)
    return assemble_output(res.results)

